# revision 1
# baseline (speedup 1.0000x reference)
"""CARAFE content-aware upsampling kernel for Trainium2 (Bass/Tile), 8 NeuronCores.

Problem (hardcoded): features [4, 256, 64, 64] f32, masks [4, 25, 128, 128] f32,
K=5, G=1, S=2 -> output [4, 256, 128, 128] f32.

Strategy
--------
Sharding: 8 cores = (batch n in 0..3) x (output-row half yh in 0..1); each core
computes out[n, :, yh*64:(yh+1)*64, :] for ALL 256 channels. The banded mask
operand depends only on (n, y), so splitting y (not channels) halves its HBM
traffic.

Compute mapping: CARAFE's per-output-pixel 25-tap weighted sum is cast as
TensorEngine matmuls contracting over the padded input-width axis wp (K=68):

  out[c, 2*h2+py, x] = sum_{hp, wp} bnd[hp, wp, kr=hp-h2, py, x] * ft[wp, hp, c]

ft is the zero-padded transposed feature map (bf16); bnd is a host-built banded
mask operand (bf16): for input row hp and tap-row kr, column (py, x) holds
mask m[kr*5+dw, 2*(hp-kr)+py, x] at partition wp = floor(x/2)+dw, else zero.

Per local input row hpl (36 rows/core): 2 stationary loads (c-halves) and up to
10 matmuls ([68, 256] moving operand) accumulating f32 into the PSUM tile
[128, 512] of output pair h2 = hpl-kr (c-half 0 in cols 0:256, half 1 in
256:512; one accumulation group per tile since PSUM zero regions are
bank-granular). A rolling window of 5 PSUM tiles stays live; completed pairs
are copied (cast to bf16) into SBUF staging on alternating DVE/ACT engines and
DMA'd out two pairs at a time on alternating SP/ACT HWDGE rings; the host
upcasts to f32.
"""

import sys

sys.path.insert(0, "/opt/trn_rl_repo")

import numpy as np
import ml_dtypes

import concourse.bacc as bacc
import concourse.mybir as mybir
from concourse import tile
from concourse import bass_utils

N, C, H, W = 4, 256, 64, 64
S = 2
KK = 5
HO, WO = H * S, W * S  # 128, 128
HP = H + KK - 1  # 68 padded rows
WP = W + KK - 1  # 68 padded cols
NCORES = 8

HPL = 36  # padded input rows per core (32 pairs + 4 tap overlap)
NPAIR = 32  # output row-pairs per core
NBLK = 18  # hpl DMA blocks of 2
BLKH = 2  # hpl rows per band DMA block
FW = KK * 2 * WO  # 1280 band cols per input row

BF16 = ml_dtypes.bfloat16


def _host_prep(features: np.ndarray, masks: np.ndarray):
    """Per-core transposed/padded features and banded mask operands."""
    # ft_g[n, wp, hp, c] = features[n, c, hp-2, wp-2]  (zero pad)
    ft_g = np.zeros((N, WP, HP, C), np.float32)
    ft_g[:, 2 : 2 + W, 2 : 2 + H, :] = features.transpose(0, 3, 2, 1)
    ft_g = ft_g.astype(BF16)

    # bnd_g[n, hp, wp, kr, py, x] = masks[n, kr*5+dw, 2*(hp-kr)+py, x]
    #   where dw = wp - floor(x/2), nonzero only for dw in [0, 5)
    bnd_g = np.zeros((N, HP, WP, KK, 2, WO), np.float32)
    st = [s // 4 for s in bnd_g.strides]  # element strides
    m6 = masks.reshape(N, KK * KK, H, 2, W, 2)
    for kr in range(KK):
        for dw in range(KK):
            base = bnd_g[:, kr:, dw:, kr, :, :]
            view = np.lib.stride_tricks.as_strided(
                base,
                shape=(N, H, 2, W, 2),
                strides=tuple(
                    4 * s
                    for s in (st[0], st[1], st[4], st[2] + 2 * st[5], st[5])
                ),
            )
            view[...] = m6[:, kr * KK + dw]
    bnd_g = bnd_g.astype(BF16)

    fts, bnds = [], []
    for i in range(NCORES):
        n, yh = divmod(i, 2)
        fts.append(np.ascontiguousarray(ft_g[n, :, yh * NPAIR : yh * NPAIR + HPL, :]))
        b = bnd_g[n, yh * NPAIR : yh * NPAIR + HPL].reshape(NBLK, BLKH, WP, FW)
        bnds.append(np.ascontiguousarray(b.transpose(0, 2, 1, 3)))
    return fts, bnds


_NC_CACHE = []


def _build_nc():
    """Build + compile the single-core Tile program (same for all 8 cores)."""
    if _NC_CACHE:
        return _NC_CACHE[0]

    nc = bacc.Bacc("TRN2", target_bir_lowering=False, debug=False)
    ft = nc.dram_tensor(
        "ft", [WP, HPL * C], mybir.dt.bfloat16, kind="ExternalInput"
    ).ap()
    bnd = nc.dram_tensor(
        "bnd", [NBLK, WP, BLKH * FW], mybir.dt.bfloat16, kind="ExternalInput"
    ).ap()
    out = nc.dram_tensor(
        "out", [C, 2 * NPAIR, WO], mybir.dt.bfloat16, kind="ExternalOutput"
    ).ap()
    outf = out.rearrange("c y x -> c (y x)")  # [256, 64*128]

    with tile.TileContext(nc) as tc:
        with (
            tc.tile_pool(name="ftp", bufs=4) as ftp,
            tc.tile_pool(name="bnp", bufs=6) as bnp,
            tc.tile_pool(name="pp", bufs=8, space="PSUM") as pp,
            tc.tile_pool(name="op", bufs=4) as op,
        ):
            psums = {}
            ft_tiles = {}
            FC = 9  # hpl rows per feature chunk tile
            for blk in range(NBLK):
                bnt = bnp.tile([WP, BLKH * FW], mybir.dt.bfloat16)
                nc.sync.dma_start(bnt[:], bnd[blk])
                if blk in (0, 2, 4, 6):
                    # interleave feature chunks between the band blocks
                    ci = blk // 2
                    fct = ftp.tile([WP, FC * C], mybir.dt.bfloat16,
                                   name="fct", tag="fct")
                    nc.scalar.dma_start(
                        fct[:], ft[:, ci * FC * C : (ci + 1) * FC * C]
                    )
                    ft_tiles[ci] = fct
                for i4 in range(BLKH):
                    hpl = BLKH * blk + i4
                    for ch in (0, 1):
                        fci, fcr = divmod(hpl, FC)
                        lhsT = ft_tiles[fci][
                            :, fcr * C + ch * 128 : fcr * C + ch * 128 + 128
                        ]
                        for kr in range(KK):
                            h2 = hpl - kr
                            if not (0 <= h2 < NPAIR):
                                continue
                            if kr == 0 and ch == 0:
                                psums[h2] = pp.tile(
                                    [128, 2 * 2 * WO], mybir.dt.float32,
                                    name="ps", tag="ps",
                                )
                            # One PSUM accumulation group per pair tile (zero
                            # regions are bank-granular): open at the first
                            # matmul (ch0/kr0), close at the last (ch1/kr4).
                            nc.tensor.matmul(
                                psums[h2][:, ch * 2 * WO : (ch + 1) * 2 * WO],
                                lhsT,
                                bnt[:, i4 * FW + kr * 2 * WO : i4 * FW + (kr + 1) * 2 * WO],
                                start=(kr == 0 and ch == 0),
                                stop=(kr == KK - 1 and ch == 1),
                            )
                    h2 = hpl - (KK - 1)
                    if 0 <= h2 < NPAIR:
                        pt = psums.pop(h2)
                        g = h2 % 2
                        if g == 0:
                            ot = op.tile([128, 2 * 512], mybir.dt.bfloat16,
                                         name="ot", tag="ot")
                            psums["ot"] = ot
                        ot = psums["ot"]
                        # staging cols: [ch, g, py*x] to keep DMA APs 3-dim
                        otv = ot.rearrange("p (ch g f) -> p ch g f", ch=2, g=2)
                        src = pt.rearrange("p (ch f) -> p ch f", ch=2)
                        if (h2 // 2) % 2 == 0:
                            nc.vector.tensor_copy(otv[:, :, g, :], src)
                        else:
                            nc.scalar.copy(otv[:, :, g, :], src)
                        if g == 1:
                            sv = ot.rearrange("p (ch gf) -> p ch gf", ch=2)
                            ov = outf.rearrange("(ch p) f -> p ch f", ch=2)
                            g0 = h2 - 1
                            deng = nc.scalar if (h2 // 2) % 2 == 0 else nc.sync
                            deng.dma_start(
                                ov[:, :, 2 * WO * g0 : 2 * WO * (g0 + 2)], sv
                            )

    nc.compile()
    _NC_CACHE.append(nc)
    return nc


def kernel(features: np.ndarray, masks: np.ndarray) -> np.ndarray:
    features = np.ascontiguousarray(features, dtype=np.float32)
    masks = np.ascontiguousarray(masks, dtype=np.float32)
    fts, bnds = _host_prep(features, masks)

    nc = _build_nc()
    in_maps = [
        {"ft": fts[i].reshape(WP, HPL * C), "bnd": bnds[i].reshape(NBLK, WP, BLKH * FW)}
        for i in range(NCORES)
    ]

    res = bass_utils.run_bass_kernel_spmd(nc, in_maps, list(range(NCORES)))

    out = np.empty((N, C, HO, WO), np.float32)
    for i in range(NCORES):
        n, yh = divmod(i, 2)
        out[n, :, yh * 2 * NPAIR : (yh + 1) * 2 * NPAIR, :] = (
            res.results[i]["out"].astype(np.float32).reshape(C, 2 * NPAIR, WO)
        )
    return out



# revision 2
# speedup vs baseline: 1.6041x; 1.6041x over previous
"""CARAFE content-aware upsampling kernel for Trainium2 (Bass/Tile), 8 NeuronCores.

Problem (hardcoded): features [4, 256, 64, 64] f32, masks [4, 25, 128, 128] f32,
K=5, G=1, S=2 -> output [4, 256, 128, 128] f32.

Strategy
--------
Sharding: 8 cores = (batch n in 0..3) x (output-row half yh in 0..1); each core
computes out[n, :, yh*64:(yh+1)*64, :] for all 256 channels.

Compute mapping: the full 25-tap weighted sum for a block of output pixels is
cast as ONE accumulation group of two TensorEngine matmuls whose contraction
axis packs (feature row, padded column window) pairs:

  block (bg, c) covers pixels (pair l2 = 4*bg+pl, py, x = 16*c+xl): 128 pixels.
  Receptive field: padded rows hp = 4*bg..4*bg+7 (two row chunks j = bg, bg+1
  of 4 rows), padded cols wp = 8*c..8*c+11 (12 wide).

  psum[ch, pix] += sum_{rl, wl} F[j][c][(rl, wl), ch] * B[bg][xy][c][(rl, wl), pix]

  F[j][c] = ft[4j+rl, 8c+wl, ch] (48 x 256 bf16, host-packed; row chunks are
  shared by adjacent blocks so HBM traffic stays low), B = host-built banded
  mask operand (48 x 128 bf16): nonzero where kr = rl-pl+4*xy and
  dw = wl-floor(xl/2) are both in [0, 5), holding masks[kr*5+dw, y, x].

Per (bg, ch-half, c-quad): one PSUM bank tile [128, 512] takes 8 matmuls (4
c-chunks x {j, j+1}), start=True once per bank (clears the whole bank's
has_written bits), stop=True on the last. DVE/ACT copy+cast psum slices into a
[128, 2048] bf16 staging tile per bg; one DMA per bg writes 8 output rows for
all 256 channels (2KB contiguous runs). The host upcasts to f32.
"""

import sys

sys.path.insert(0, "/opt/trn_rl_repo")

import numpy as np
import ml_dtypes

import concourse.bacc as bacc
import concourse.mybir as mybir
from concourse import tile
from concourse import bass_utils

N, C, H, W = 4, 256, 64, 64
S = 2
KK = 5
HO, WO = H * S, W * S  # 128, 128
NCORES = 8

NBG = 8   # row-pair groups per core (4 pairs = 8 output rows each)
NCH = 8   # x chunks per core (16 output cols each)
NJ = 9    # 4-row feature chunks per core (36 padded rows)
RW = 48   # contraction partitions per matmul: 4 rows x 12 wl
FTF = NJ * NCH * C   # 18432 ftb free elems
BNF = NBG * 2 * NCH * 128  # 16384 bnd free elems

BF16 = ml_dtypes.bfloat16


def _bnd_indices():
    bg = np.arange(NBG).reshape(NBG, 1, 1, 1, 1, 1, 1, 1)
    xy = np.arange(2).reshape(1, 2, 1, 1, 1, 1, 1, 1)
    c = np.arange(NCH).reshape(1, 1, NCH, 1, 1, 1, 1, 1)
    rl = np.arange(4).reshape(1, 1, 1, 4, 1, 1, 1, 1)
    wl = np.arange(12).reshape(1, 1, 1, 1, 12, 1, 1, 1)
    pl = np.arange(4).reshape(1, 1, 1, 1, 1, 4, 1, 1)
    py = np.arange(2).reshape(1, 1, 1, 1, 1, 1, 2, 1)
    xl = np.arange(16).reshape(1, 1, 1, 1, 1, 1, 1, 16)
    kr = rl - pl + 4 * xy
    dw = wl - xl // 2
    valid = (kr >= 0) & (kr <= 4) & (dw >= 0) & (dw <= 4)
    chan = np.clip(kr, 0, 4) * KK + np.clip(dw, 0, 4)
    ylo = 8 * bg + 2 * pl + py
    x = 16 * c + xl
    return np.broadcast_arrays(chan, ylo, x, valid)


_CHAN, _YLO, _X, _VALID = _bnd_indices()


def _host_prep(features: np.ndarray, masks: np.ndarray):
    """Per-core packed feature chunks and banded mask operands."""
    ftg = np.zeros((N, H + 4, W + 4, C), np.float32)
    ftg[:, 2 : 2 + H, 2 : 2 + W, :] = features.transpose(0, 2, 3, 1)

    fts, bnds = [], []
    for i in range(NCORES):
        n, yh = divmod(i, 2)
        flp = ftg[n, 32 * yh : 32 * yh + 36]  # [36, 68, C]
        fj = flp.reshape(NJ, 4, W + 4, C)
        s = fj.strides
        fw = np.lib.stride_tricks.as_strided(
            fj, shape=(NJ, 4, NCH, 12, C), strides=(s[0], s[1], 8 * s[2], s[2], s[3])
        )
        ftb = np.ascontiguousarray(fw.transpose(1, 3, 0, 2, 4)).reshape(RW, FTF)
        fts.append(ftb.astype(BF16))

        m = masks[n, :, 64 * yh : 64 * yh + 64, :]
        vals = np.where(_VALID, m[_CHAN, _YLO, _X], np.float32(0.0))
        b = np.ascontiguousarray(vals.transpose(3, 4, 0, 1, 2, 5, 6, 7)).reshape(RW, BNF)
        bnds.append(b.astype(BF16))
    return fts, bnds


_NC_CACHE = []


def _build_nc():
    """Build + compile the single-core Tile program (same for all 8 cores)."""
    if _NC_CACHE:
        return _NC_CACHE[0]

    nc = bacc.Bacc("TRN2", target_bir_lowering=False, debug=False)
    ftb = nc.dram_tensor("ftb", [RW, FTF], mybir.dt.bfloat16, kind="ExternalInput").ap()
    bnd = nc.dram_tensor("bnd", [RW, BNF], mybir.dt.bfloat16, kind="ExternalInput").ap()
    out = nc.dram_tensor("out", [C, HO // 2 * WO], mybir.dt.bfloat16, kind="ExternalOutput").ap()
    ov = out.rearrange("(g p) f -> p g f", g=2)  # [128, 2, 8192]

    with tile.TileContext(nc) as tc:
        with (
            tc.tile_pool(name="ftp", bufs=1) as ftp,
            tc.tile_pool(name="bnp", bufs=1) as bnp,
            tc.tile_pool(name="pp", bufs=8, space="PSUM") as pp,
            tc.tile_pool(name="stp", bufs=3) as stp,
        ):
            ft = ftp.tile([RW, FTF], mybir.dt.bfloat16)
            bn = bnp.tile([RW, BNF], mybir.dt.bfloat16)
            # Input DMAs, ordered so bg0's operands land first.
            nc.sync.dma_start(ft[:, 0:4096], ftb[:, 0:4096])          # j 0-1
            nc.sync.dma_start(bn[:, 0:2048], bnd[:, 0:2048])          # bg 0
            nc.sync.dma_start(ft[:, 4096:12288], ftb[:, 4096:12288])  # j 2-5
            nc.sync.dma_start(bn[:, 2048:6144], bnd[:, 2048:6144])    # bg 1-2
            nc.sync.dma_start(ft[:, 12288:18432], ftb[:, 12288:18432])  # j 6-8
            nc.sync.dma_start(bn[:, 6144:10240], bnd[:, 6144:10240])  # bg 3-4
            nc.sync.dma_start(bn[:, 10240:16384], bnd[:, 10240:16384])  # bg 5-7

            for bg in range(NBG):
                st = stp.tile([128, 2 * 8 * WO], mybir.dt.bfloat16, name="st", tag="st")
                # st free layout: (ch2, y=2*pl+py: 8, x=16*c+xl: 128)
                stv = st.rearrange(
                    "p (ch pl py c xl) -> p ch c pl py xl", ch=2, pl=4, py=2, c=NCH
                )
                for ch in range(2):
                    for half in range(2):
                        ps = pp.tile([128, 512], mybir.dt.float32, name="ps", tag="ps")
                        for cq in range(4):
                            ci = half * 4 + cq
                            for xyi in range(2):
                                j = bg + xyi
                                fo = (j * NCH + ci) * C + ch * 128
                                bo = bg * 2048 + xyi * 1024 + ci * 128
                                nc.tensor.matmul(
                                    ps[:, cq * 128 : (cq + 1) * 128],
                                    ft[:, fo : fo + 128],
                                    bn[:, bo : bo + 128],
                                    start=(cq == 0 and xyi == 0),
                                    stop=(cq == 3 and xyi == 1),
                                )
                        psv = ps.rearrange("p (cq pl py xl) -> p cq pl py xl",
                                           cq=4, pl=4, py=2)
                        eng = nc.vector.tensor_copy if (ch + half) % 2 == 0 else nc.scalar.copy
                        for cq in range(4):
                            ci = half * 4 + cq
                            eng(stv[:, ch, ci], psv[:, cq])
                nc.sync.dma_start(
                    ov[:, :, bg * 1024 : (bg + 1) * 1024],
                    st.rearrange("p (g f) -> p g f", g=2),
                )

    nc.compile()
    _NC_CACHE.append(nc)
    return nc


def kernel(features: np.ndarray, masks: np.ndarray) -> np.ndarray:
    features = np.ascontiguousarray(features, dtype=np.float32)
    masks = np.ascontiguousarray(masks, dtype=np.float32)
    fts, bnds = _host_prep(features, masks)

    nc = _build_nc()
    in_maps = [{"ftb": fts[i], "bnd": bnds[i]} for i in range(NCORES)]

    res = bass_utils.run_bass_kernel_spmd(nc, in_maps, list(range(NCORES)))

    outv = np.empty((N, C, HO, WO), np.float32)
    for i in range(NCORES):
        n, yh = divmod(i, 2)
        outv[n, :, yh * 64 : (yh + 1) * 64, :] = (
            res.results[i]["out"].astype(np.float32).reshape(C, 64, WO)
        )
    return outv


# revision 3
# speedup vs baseline: 1.9177x; 1.1955x over previous
"""CARAFE content-aware upsampling kernel for Trainium2 (Bass/Tile), 8 NeuronCores.

Problem (hardcoded): features [4, 256, 64, 64] f32, masks [4, 25, 128, 128] f32,
K=5, G=1, S=2 -> output [4, 256, 128, 128] f32.

Strategy
--------
Sharding: 8 cores = (batch n in 0..3) x (output-row half yh in 0..1); each core
computes out[n, :, yh*64:(yh+1)*64, :] for all 256 channels.

Compute mapping: the full 25-tap weighted sum for a block of output pixels is
cast as ONE accumulation group of two TensorEngine matmuls whose contraction
axis packs (feature row, padded column window) pairs:

  block (bg, c) covers pixels (pair l2 = 4*bg+pl, py, x = 16*c+xl): 128 pixels.
  Receptive field: padded rows hp = 4*bg..4*bg+7 (two row chunks j = bg, bg+1
  of 4 rows), padded cols wp = 8*c..8*c+11 (12 wide).

  psum[ch, pix] += sum_{rl, wl} F[j][c][(rl, wl), ch] * B[bg][xy][c][(rl, wl), pix]

  F[j][c] = ft[4j+rl, 8c+wl, ch] (48 x 256 bf16, host-packed; row chunks are
  shared by adjacent blocks so HBM traffic stays low), B = host-built banded
  mask operand (48 x 128 bf16): nonzero where kr = rl-pl+4*xy and
  dw = wl-floor(xl/2) are both in [0, 5), holding masks[kr*5+dw, y, x].

Per (bg, ch-half, c-quad): one PSUM bank tile [128, 512] takes 8 matmuls (4
c-chunks x {j, j+1}), start=True once per bank (clears the whole bank's
has_written bits), stop=True on the last. DVE/ACT copy+cast psum slices into a
[128, 2048] bf16 staging tile per bg; one DMA per bg writes 8 output rows for
all 256 channels (2KB contiguous runs). The host upcasts to f32.
"""

import sys

sys.path.insert(0, "/opt/trn_rl_repo")

import numpy as np
import ml_dtypes

import concourse.bacc as bacc
import concourse.mybir as mybir
from concourse import tile
from concourse import bass_utils

N, C, H, W = 4, 256, 64, 64
S = 2
KK = 5
HO, WO = H * S, W * S  # 128, 128
NCORES = 8

NBG = 8   # row-pair groups per core (4 pairs = 8 output rows each)
NCH = 8   # x chunks per core (16 output cols each)
NJ = 9    # 4-row feature chunks per core (36 padded rows)
RW = 48   # contraction partitions per matmul: 4 rows x 12 wl
FTF = NJ * NCH * C   # 18432 ftb free elems
BNF = NBG * 2 * NCH * 128  # 16384 bnd free elems

BF16 = ml_dtypes.bfloat16


def _bnd_indices():
    bg = np.arange(NBG).reshape(NBG, 1, 1, 1, 1, 1, 1, 1)
    xy = np.arange(2).reshape(1, 2, 1, 1, 1, 1, 1, 1)
    c = np.arange(NCH).reshape(1, 1, NCH, 1, 1, 1, 1, 1)
    rl = np.arange(4).reshape(1, 1, 1, 4, 1, 1, 1, 1)
    wl = np.arange(12).reshape(1, 1, 1, 1, 12, 1, 1, 1)
    pl = np.arange(4).reshape(1, 1, 1, 1, 1, 4, 1, 1)
    py = np.arange(2).reshape(1, 1, 1, 1, 1, 1, 2, 1)
    xl = np.arange(16).reshape(1, 1, 1, 1, 1, 1, 1, 16)
    kr = rl - pl + 4 * xy
    dw = wl - xl // 2
    valid = (kr >= 0) & (kr <= 4) & (dw >= 0) & (dw <= 4)
    chan = np.clip(kr, 0, 4) * KK + np.clip(dw, 0, 4)
    ylo = 8 * bg + 2 * pl + py
    x = 16 * c + xl
    return np.broadcast_arrays(chan, ylo, x, valid)


_CHAN, _YLO, _X, _VALID = _bnd_indices()


def _host_prep(features: np.ndarray, masks: np.ndarray):
    """Per-core packed feature chunks and banded mask operands."""
    ftg = np.zeros((N, H + 4, W + 4, C), np.float32)
    ftg[:, 2 : 2 + H, 2 : 2 + W, :] = features.transpose(0, 2, 3, 1)

    fts, bnds = [], []
    for i in range(NCORES):
        n, yh = divmod(i, 2)
        flp = ftg[n, 32 * yh : 32 * yh + 36]  # [36, 68, C]
        fj = flp.reshape(NJ, 4, W + 4, C)
        s = fj.strides
        fw = np.lib.stride_tricks.as_strided(
            fj, shape=(NJ, 4, NCH, 12, C), strides=(s[0], s[1], 8 * s[2], s[2], s[3])
        )
        ftb = np.ascontiguousarray(fw.transpose(1, 3, 0, 2, 4)).reshape(RW, FTF)
        fts.append(ftb.astype(BF16))

        m = masks[n, :, 64 * yh : 64 * yh + 64, :]
        vals = np.where(_VALID, m[_CHAN, _YLO, _X], np.float32(0.0))
        b = np.ascontiguousarray(vals.transpose(3, 4, 0, 1, 2, 5, 6, 7)).reshape(RW, BNF)
        bnds.append(b.astype(BF16))
    return fts, bnds


_NC_CACHE = []


def _build_nc():
    """Build + compile the single-core Tile program (same for all 8 cores)."""
    if _NC_CACHE:
        return _NC_CACHE[0]

    nc = bacc.Bacc("TRN2", target_bir_lowering=False, debug=False)
    ftb = nc.dram_tensor("ftb", [RW, FTF], mybir.dt.bfloat16, kind="ExternalInput").ap()
    bnd = nc.dram_tensor("bnd", [RW, BNF], mybir.dt.bfloat16, kind="ExternalInput").ap()
    out = nc.dram_tensor("out", [C, HO // 2 * WO], mybir.dt.bfloat16, kind="ExternalOutput").ap()
    ov = out.rearrange("(g p) f -> p g f", g=2)  # [128, 2, 8192]

    with tile.TileContext(nc) as tc:
        with (
            tc.tile_pool(name="ftp", bufs=1) as ftp,
            tc.tile_pool(name="bnp", bufs=1) as bnp,
            tc.tile_pool(name="pp", bufs=8, space="PSUM") as pp,
            tc.tile_pool(name="stp", bufs=3) as stp,
        ):
            ft = ftp.tile([RW, FTF], mybir.dt.bfloat16)
            bn = bnp.tile([RW, BNF], mybir.dt.bfloat16)
            ftj = ft.rearrange("p (j f) -> p j f", j=NJ)
            fdj = ftb.rearrange("p (j f) -> p j f", j=NJ)
            bnb = bn.rearrange("p (bg xy f) -> p bg xy f", bg=NBG, xy=2)
            bdb = bnd.rearrange("p (bg xy f) -> p bg xy f", bg=NBG, xy=2)
            # Input DMAs, ordered so bg0/ch0/half0's operands land first.
            nc.sync.dma_start(bnb[:, 0, :, 0:512], bdb[:, 0, :, 0:512])
            nc.sync.dma_start(ftj[:, 0:2, 0:1024], fdj[:, 0:2, 0:1024])
            nc.sync.dma_start(bnb[:, 0, :, 512:1024], bdb[:, 0, :, 512:1024])
            nc.sync.dma_start(ftj[:, 0:2, 1024:2048], fdj[:, 0:2, 1024:2048])
            nc.sync.dma_start(ft[:, 4096:12288], ftb[:, 4096:12288])  # j 2-5
            nc.sync.dma_start(bn[:, 2048:6144], bnd[:, 2048:6144])    # bg 1-2
            nc.sync.dma_start(ft[:, 12288:18432], ftb[:, 12288:18432])  # j 6-8
            nc.sync.dma_start(bn[:, 6144:10240], bnd[:, 6144:10240])  # bg 3-4
            nc.sync.dma_start(bn[:, 10240:16384], bnd[:, 10240:16384])  # bg 5-7

            for bg in range(NBG):
                st = stp.tile([128, 2 * 8 * WO], mybir.dt.bfloat16, name="st", tag="st")
                # st free layout: (ch2, y = 2*pl+py: 8, x = 64*half+16*cq+xl: 128)
                stv = st.rearrange(
                    "p (ch pl py xh xx) -> p ch pl py xh xx", ch=2, pl=4, py=2, xh=2
                )
                for ch in range(2):
                    for half in range(2):
                        ps = pp.tile([128, 512], mybir.dt.float32, name="ps", tag="ps")
                        # psum free layout: (pl, py, cq, xl)
                        psv = ps.rearrange("p (pl py cq xl) -> p pl py cq xl",
                                           pl=4, py=2, cq=4)
                        for cq in range(4):
                            ci = half * 4 + cq
                            for xyi in range(2):
                                j = bg + xyi
                                fo = (j * NCH + ci) * C + ch * 128
                                bo = bg * 2048 + xyi * 1024 + ci * 128
                                nc.tensor.matmul(
                                    psv[:, :, :, cq, :],
                                    ft[:, fo : fo + 128],
                                    bn[:, bo : bo + 128],
                                    start=(cq == 0 and xyi == 0),
                                    stop=(cq == 3 and xyi == 1),
                                )
                        src = ps.rearrange("p (pl py xx) -> p pl py xx", pl=4, py=2)
                        if (ch + half) % 2 == 0:
                            nc.vector.tensor_copy(stv[:, ch, :, :, half, :], src)
                        else:
                            nc.scalar.copy(stv[:, ch, :, :, half, :], src)
                    nc.sync.dma_start(
                        ov[:, ch, bg * 1024 : (bg + 1) * 1024],
                        st[:, ch * 1024 : (ch + 1) * 1024],
                    )

    nc.compile()
    _NC_CACHE.append(nc)
    return nc


def kernel(features: np.ndarray, masks: np.ndarray) -> np.ndarray:
    features = np.ascontiguousarray(features, dtype=np.float32)
    masks = np.ascontiguousarray(masks, dtype=np.float32)
    fts, bnds = _host_prep(features, masks)

    nc = _build_nc()
    in_maps = [{"ftb": fts[i], "bnd": bnds[i]} for i in range(NCORES)]

    res = bass_utils.run_bass_kernel_spmd(nc, in_maps, list(range(NCORES)))

    outv = np.empty((N, C, HO, WO), np.float32)
    for i in range(NCORES):
        n, yh = divmod(i, 2)
        outv[n, :, yh * 64 : (yh + 1) * 64, :] = (
            res.results[i]["out"].astype(np.float32).reshape(C, 64, WO)
        )
    return outv
